# revision 5
# baseline (speedup 1.0000x reference)
"""kNN hypergraph kernel for Trainium2 (8 NeuronCores, Bass/Tile).

Problem: x [16, 256, 768] f32, k=16.
  flat = x.reshape(4096, 768)
  d2[i,j] = |flat_i - flat_j|^2 ; idx = 16 nearest (incl self)
  hypergraph[i, idx[i,:]] = 1 ; out[b,s,t] = sum_b2 hg[b*256+s, b2*256+t]
Output: [16, 256, 256] f32 (per-row histogram of neighbor_index % 256).

Strategy (row-sharded across 8 cores, 512 rows each):
  - Rank rows by s[i,j] = <x_i,x_j> - |x_j|^2/2 (uniform halving of
    2<x_i,x_j> - |x_j|^2; the per-row constant sq_i does not change the
    ranking). The 16 NN are the 16 LARGEST s per row.
  - Exact-match numerics via a hi/lo fp16+fp8 split at a 2^12 PSUM scale:
      hh:    (2^6 hi)^T (2^6 hi)          fp16, 6 K-tiles of 128
      cross: (2^12 lo8)^T hi8 + hi8^T (2^12 lo8)
                                          fp8 e4m3 DoubleRow, 2x3 K-tiles
      sq:    (2^7 ones, K=2)^T (2^4 [-sq_h; -sq_l])  fp16, one matmul
    13 matmul-tiles per (row-tile, col-block); every matmul costs the
    same 216ns (512 output cols at 1 col/cycle), so slots are the PE
    currency.
  - DMA layout: streamed tensors are packed partition-major with >=16KB
    contiguous per partition so the DMA queue runs at line rate
    (~437 GB/s measured) instead of descriptor-bound (~112 GB/s at
    2.7KB packets).
  - Top-16 per row: per 256-column chunk one DVE max8 -> chunk top-8;
    combine -> sigma = 16th largest; mask (s >= sigma) fused with the
    first fold, then binary-tree adds fold the 16 blocks of 256.
  - Engine split: PE matmuls; Act drains PSUM->SBUF; DVE topk/mask/
    folds; GpSimd does the rt0 backfill adds.
  - Phasing: rt0 hh+sq paced by the rh16 stream (closed early, partial
    sums drained); rt1 runs fully fused while the fp8 stream lands;
    rt0's cross then backfills into fresh PSUM groups (Act drain +
    GpSimd add); rt2/rt3 run fully fused, rt3 n-outer so its drain/topk
    chase per column block, shrinking the tail.
"""

import os

import numpy as np

B, S, D = 16, 256, 768
N = B * S            # 4096 points
NCORES = 8
M = N // NCORES      # 512 rows per core
KT = 6               # fp16 K tiles of 128 (768 features)
KT8 = 3              # fp8 DoubleRow K tiles of 256
NT = N // 512        # 8 moving tiles of 512 columns
RT = M // 128        # 4 row-tiles of 128 per core
NEG = -3.0e38        # sentinel: far below any real s value

_cache = {}


def _build():
    import concourse.mybir as mybir
    import concourse.tile as tile
    from concourse import bacc

    f32 = mybir.dt.float32
    f16 = mybir.dt.float16
    bf16 = mybir.dt.bfloat16
    f8 = mybir.dt.float8e4
    DR = mybir.MatmulPerfMode.DoubleRow

    nc = bacc.Bacc("TRN2", target_bir_lowering=False, debug=False,
                   num_devices=NCORES)

    rh16_d = nc.dram_tensor("rh16", [128, KT, N], f16, kind="ExternalInput")
    rh8_d = nc.dram_tensor("rh8", [128, KT8, 2, 2, N], f8,
                           kind="ExternalInput")
    lh16_d = nc.dram_tensor("lh16", [128, KT, M], f16, kind="ExternalInput")
    lh8_d = nc.dram_tensor("lh8", [128, KT8, 2, 2, M], f8,
                           kind="ExternalInput")
    sq_d = nc.dram_tensor("sqrows", [2, N], f16, kind="ExternalInput")
    out_d = nc.dram_tensor("out", [M, S], f32, kind="ExternalOutput")

    with tile.TileContext(nc) as tc:
        with (
            tc.tile_pool(name="weights", bufs=1) as wpool,
            tc.tile_pool(name="s", bufs=2) as spool,
            tc.tile_pool(name="s01", bufs=1) as spool0,
            tc.tile_pool(name="tmp", bufs=2) as tpool,
            tc.tile_pool(name="mask", bufs=2) as mpool,
            tc.tile_pool(name="m8", bufs=2) as m8pool,
            tc.tile_pool(name="c8", bufs=4) as c8pool,
            tc.tile_pool(name="outp", bufs=4) as opool,
            tc.tile_pool(name="psum", bufs=8, space="PSUM") as psum,
        ):
            sq_sb = wpool.tile([2, N], f16, tag="sq", name="sq")
            nc.sync.dma_start(out=sq_sb, in_=sq_d[:, :])
            lh16 = wpool.tile([128, KT, M], f16, tag="lh16", name="lh16")
            nc.sync.dma_start(out=lh16, in_=lh16_d[:, :, :])
            lh8 = wpool.tile([128, KT8, 2, 2, M], f8, tag="lh8", name="lh8")
            nc.sync.dma_start(out=lh8, in_=lh8_d[:, :, :, :, :])
            # big-row streams: 16KB contiguous per partition per DMA
            rh16 = wpool.tile([128, KT, N], f16, tag="rh16", name="rh16")
            for kc in range(KT // 2):
                nc.sync.dma_start(out=rh16[:, 2 * kc:2 * kc + 2, :],
                                  in_=rh16_d[:, 2 * kc:2 * kc + 2, :])
            rh8 = wpool.tile([128, KT8, 2, 2, N], f8, tag="rh8", name="rh8")
            for ki in range(KT8):
                nc.sync.dma_start(out=rh8[:, ki], in_=rh8_d[:, ki])
            ones = wpool.tile([2, 128], f16, tag="ones", name="ones")
            nc.vector.memset(ones, 128.0)

            def hh(ps, rt, ki, n, start, stop=False):
                nc.tensor.matmul(
                    ps[n][:, :], lh16[:, ki, rt * 128:(rt + 1) * 128],
                    rh16[:, ki, n * 512:(n + 1) * 512],
                    start=start, stop=stop)

            def cross(ps, rt, ki, t, n, start=False, stop=False):
                # t=0: lo_i x hi_j ; t=1: hi_i x lo_j
                nc.tensor.matmul(
                    ps[n][:, :], lh8[:, ki, t, :, rt * 128:(rt + 1) * 128],
                    rh8[:, ki, 1 - t, :, n * 512:(n + 1) * 512],
                    start=start, stop=stop, perf_mode=DR)

            def sq_close(ps, n, stop=True):
                nsl = slice(n * 512, (n + 1) * 512)
                nc.tensor.matmul(ps[n][:, :], ones, sq_sb[:, nsl],
                                 start=False, stop=stop)

            def topk(s_sb, m8, n):
                for h in range(2):
                    cs = slice(n * 512 + h * 256, n * 512 + (h + 1) * 256)
                    nc.vector.max(out=m8[:, n * 16 + h * 8:
                                         n * 16 + (h + 1) * 8],
                                  in_=s_sb[:, cs])

            def epilogue(s_sb, m8, rt):
                c8 = c8pool.tile([128, 8], f32, tag="c8", name="c8")
                m8x = m8pool.tile([128, 16 * 8], f32, tag="m8x", name="m8x")
                d8 = c8pool.tile([128, 8], f32, tag="d8", name="d8")
                nc.vector.max(out=c8, in_=m8)
                nc.vector.match_replace(out=m8x, in_to_replace=c8,
                                        in_values=m8, imm_value=NEG)
                nc.vector.max(out=d8, in_=m8x)
                sigma = d8[:, 7:8]

                H = N // 2
                mask = mpool.tile([128, H], bf16, tag="mask", name="mask")
                nc.vector.tensor_scalar(mask, s_sb[:, :H], sigma, None,
                                        op0=mybir.AluOpType.is_ge)
                nc.vector.scalar_tensor_tensor(
                    out=mask, in0=s_sb[:, H:], scalar=sigma, in1=mask,
                    op0=mybir.AluOpType.is_ge, op1=mybir.AluOpType.add)
                w = H // 2
                while w > S:
                    nc.vector.tensor_add(mask[:, :w], mask[:, :w],
                                         mask[:, w:2 * w])
                    w //= 2
                o = opool.tile([128, S], f32, tag="o", name="o")
                nc.vector.tensor_add(o, mask[:, :S], mask[:, S:2 * S])
                nc.sync.dma_start(
                    out=out_d[rt * 128:(rt + 1) * 128, :], in_=o)

            # ---- phase 1: rt0 hh+sq paced by the rh16 stream; closed
            # early so rt1 can take the PSUM banks; cross backfills later.
            s0 = spool0.tile([128, N], f32, tag="s00", name="s00")
            ps = [psum.tile([128, 512], f32, tag="ps", name=f"ps{n}")
                  for n in range(NT)]
            for ki in range(KT):
                for n in range(NT):
                    hh(ps, 0, ki, n, start=(ki == 0))
            for n in range(NT):
                sq_close(ps, n)
                nc.scalar.mul(s0[:, n * 512:(n + 1) * 512], ps[n][:, :], 1.0)

            def full_rt(rt, s_sb, m8, n_outer=False):
                ps = [psum.tile([128, 512], f32, tag="ps", name=f"ps{n}")
                      for n in range(NT)]
                if n_outer:
                    for n in range(NT):
                        for ki in range(KT):
                            hh(ps, rt, ki, n, start=(ki == 0))
                        for ki in range(KT8):
                            for t in range(2):
                                cross(ps, rt, ki, t, n)
                        sq_close(ps, n)
                        nc.scalar.mul(s_sb[:, n * 512:(n + 1) * 512],
                                      ps[n][:, :], 1.0)
                        topk(s_sb, m8, n)
                else:
                    for ki in range(KT):
                        for n in range(NT):
                            hh(ps, rt, ki, n, start=(ki == 0))
                    for ki in range(KT8):
                        for t in range(2):
                            for n in range(NT):
                                cross(ps, rt, ki, t, n)
                    for n in range(NT):
                        sq_close(ps, n)
                        nc.scalar.mul(s_sb[:, n * 512:(n + 1) * 512],
                                      ps[n][:, :], 1.0)
                        topk(s_sb, m8, n)
                epilogue(s_sb, m8, rt)

            # ---- phase 2: rt1 fully fused (fp8 stream lands underneath)
            s1 = spool.tile([128, N], f32, tag="s", name="s_sb")
            m81 = m8pool.tile([128, 16 * 8], f32, tag="m8", name="m8")
            full_rt(1, s1, m81)

            # ---- phase 3: rt0 cross backfill; Act drains, GpSimd adds,
            # topk + epilogue chase under rt2's matmuls
            m80 = m8pool.tile([128, 16 * 8], f32, tag="m8", name="m8")
            ps = [psum.tile([128, 512], f32, tag="ps", name=f"ps{n}")
                  for n in range(NT)]
            for ki in range(KT8):
                for t in range(2):
                    for n in range(NT):
                        cross(ps, 0, ki, t, n, start=(ki == 0 and t == 0),
                              stop=(ki == KT8 - 1 and t == 1))
            for n in range(NT):
                nsl = slice(n * 512, (n + 1) * 512)
                tmp = tpool.tile([128, 512], f32, tag="tmp", name="tmp")
                nc.scalar.mul(tmp, ps[n][:, :], 1.0)
                nc.gpsimd.tensor_add(s0[:, nsl], s0[:, nsl], tmp)
                topk(s0, m80, n)
            epilogue(s0, m80, 0)

            # ---- phase 4/5: rt2 fused; rt3 fused n-outer (short tail)
            s2 = spool.tile([128, N], f32, tag="s", name="s_sb")
            m82 = m8pool.tile([128, 16 * 8], f32, tag="m8", name="m8")
            full_rt(2, s2, m82)
            s3 = spool.tile([128, N], f32, tag="s", name="s_sb")
            m83 = m8pool.tile([128, 16 * 8], f32, tag="m8", name="m8")
            full_rt(3, s3, m83, n_outer=True)

    nc.compile()
    return nc


def _prep_inputs(x):
    import ml_dtypes
    f8 = ml_dtypes.float8_e4m3

    flat = np.asarray(x, dtype=np.float32).reshape(N, D)
    sq = (flat.astype(np.float64) ** 2).sum(1)

    hi = flat.astype(np.float16)
    lo = (flat - hi.astype(np.float32)).astype(np.float16)

    rh16 = (hi.astype(np.float32) * 64.0).astype(np.float16)  # [N, D]
    hi8 = hi.astype(f8)
    lo8 = (lo.astype(np.float32) * 4096.0).astype(f8)

    # -16*sq rows (ones row is 2^7 -> product -sq*2^11)
    assert np.abs(sq).max() * 16.0 < 65000.0
    nsq_h = (-16.0 * sq).astype(np.float16)
    nsq_l = (-16.0 * sq - nsq_h.astype(np.float64)).astype(np.float16)
    sqrows = np.ascontiguousarray(np.stack([nsq_h, nsq_l]))  # [2, N]

    # [128, KT, N]: [p, ki, j] = hi*64 [j, ki*128+p]
    r16 = np.ascontiguousarray(rh16.T.reshape(KT, 128, N).transpose(1, 0, 2))
    # [128, KT8, 2, 2, N]: [p, ki, t, sub, j] = (lo8,hi8)[t][j, ki*256+sub*128+p]
    l8 = lo8.T.reshape(KT8, 2, 128, N)
    h8 = hi8.T.reshape(KT8, 2, 128, N)
    r8 = np.ascontiguousarray(
        np.stack([l8, h8], axis=1).transpose(3, 0, 1, 2, 4))

    in_maps = []
    for c in range(NCORES):
        rsl = slice(c * M, (c + 1) * M)
        in_maps.append({
            "rh16": r16, "rh8": r8, "sqrows": sqrows,
            "lh16": np.ascontiguousarray(r16[:, :, rsl]),
            "lh8": np.ascontiguousarray(r8[:, :, :, :, rsl]),
        })
    return in_maps


def kernel(x, k):
    assert int(k) == 16
    in_maps = _prep_inputs(x)

    if "nc" not in _cache:
        _cache["nc"] = _build()
    nc = _cache["nc"]

    from concourse.bass_utils import run_bass_kernel_spmd
    trace = bool(os.environ.get("KNN_TRACE"))
    if trace:
        try:
            from antenv.axon_hooks import get_axon_ntff_profile_hook
        except ImportError:
            trace = False
        else:
            trace = get_axon_ntff_profile_hook() is not None
    res = run_bass_kernel_spmd(nc, in_maps, core_ids=list(range(NCORES)),
                               trace=trace)
    _cache["res"] = res
    if trace and res.exec_time_ns is not None:
        print(f"HW exec time: {res.exec_time_ns} ns")
        _cache["exec_time_ns"] = res.exec_time_ns

    out = np.concatenate([r["out"] for r in res.results], axis=0)
    return out.reshape(B, S, S)


# revision 6
# speedup vs baseline: 1.0373x; 1.0373x over previous
"""kNN hypergraph kernel for Trainium2 (8 NeuronCores, Bass/Tile).

Problem: x [16, 256, 768] f32, k=16.
  flat = x.reshape(4096, 768)
  d2[i,j] = |flat_i - flat_j|^2 ; idx = 16 nearest (incl self)
  hypergraph[i, idx[i,:]] = 1 ; out[b,s,t] = sum_b2 hg[b*256+s, b2*256+t]
Output: [16, 256, 256] f32 (per-row histogram of neighbor_index % 256).

Strategy (row-sharded across 8 cores, 512 rows each):
  - Rank rows by s[i,j] = <x_i,x_j> - |x_j|^2/2 (uniform halving of
    2<x_i,x_j> - |x_j|^2; the per-row constant sq_i does not change the
    ranking). The 16 NN are the 16 LARGEST s per row.
  - Exact-match numerics via a hi/lo fp16+fp8 split at a 2^12 PSUM scale:
      hh:    (2^6 hi)^T (2^6 hi)          fp16, 6 K-tiles of 128
      cross: (2^12 lo8)^T hi8 + hi8^T (2^12 lo8)
                                          fp8 e4m3 DoubleRow, 2x3 K-tiles
    12 matmul-tiles per (row-tile, col-block); every matmul costs the
    same 216ns (512 output cols at 1 col/cycle), so slots are the PE
    currency. The -sq*2^11 term is added during the PSUM->SBUF drain
    (DVE scalar_tensor_tensor against a broadcast tile built once at
    startup by eight K=2 ones x [-16sq_h; -16sq_l] matmuls).
  - DMA layout: streamed tensors are packed partition-major with >=8KB
    contiguous per partition so the DMA queue runs at line rate
    (~420 GB/s measured) instead of descriptor-bound (~112 GB/s at
    2.7KB packets); per-K-tile chunks keep the first matmul start early.
  - Top-16 per row: per 256-column chunk one DVE max8 -> chunk top-8;
    combine -> sigma = 16th largest; mask (s >= sigma) fused with the
    first fold, then binary-tree adds fold the 16 blocks of 256.
  - Phasing: rt0 hh paced by the rh16 stream (closed early, partial
    sums drained); rt1 runs fully fused while the fp8 stream lands;
    rt0's cross then backfills into fresh PSUM groups (Act drain +
    GpSimd add); rt2/rt3 run fully fused, rt3 n-outer so its drain/topk
    chase per column block, shrinking the tail.
"""

import os

import numpy as np

B, S, D = 16, 256, 768
N = B * S            # 4096 points
NCORES = 8
M = N // NCORES      # 512 rows per core
KT = 6               # fp16 K tiles of 128 (768 features)
KT8 = 3              # fp8 DoubleRow K tiles of 256
NT = N // 512        # 8 moving tiles of 512 columns
RT = M // 128        # 4 row-tiles of 128 per core
NEG = -3.0e38        # sentinel: far below any real s value

_cache = {}


def _build():
    import concourse.mybir as mybir
    import concourse.tile as tile
    from concourse import bacc

    f32 = mybir.dt.float32
    f16 = mybir.dt.float16
    bf16 = mybir.dt.bfloat16
    f8 = mybir.dt.float8e4
    DR = mybir.MatmulPerfMode.DoubleRow

    nc = bacc.Bacc("TRN2", target_bir_lowering=False, debug=False,
                   num_devices=NCORES)

    rh16_d = nc.dram_tensor("rh16", [128, KT, N], f16, kind="ExternalInput")
    rh8_d = nc.dram_tensor("rh8", [128, KT8, 2, 2, N], f8,
                           kind="ExternalInput")
    lh16_d = nc.dram_tensor("lh16", [128, KT, M], f16, kind="ExternalInput")
    lh8_d = nc.dram_tensor("lh8", [128, KT8, 2, 2, M], f8,
                           kind="ExternalInput")
    sq_d = nc.dram_tensor("sqrows", [2, N], f16, kind="ExternalInput")
    out_d = nc.dram_tensor("out", [M, S], f32, kind="ExternalOutput")

    with tile.TileContext(nc) as tc:
        with (
            tc.tile_pool(name="weights", bufs=1) as wpool,
            tc.tile_pool(name="s", bufs=2) as spool,
            tc.tile_pool(name="s01", bufs=1) as spool0,
            tc.tile_pool(name="tmp", bufs=2) as tpool,
            tc.tile_pool(name="mask", bufs=2) as mpool,
            tc.tile_pool(name="m8", bufs=2) as m8pool,
            tc.tile_pool(name="c8", bufs=4) as c8pool,
            tc.tile_pool(name="outp", bufs=4) as opool,
            tc.tile_pool(name="psum", bufs=8, space="PSUM") as psum,
        ):
            # DMA order tuned for earliest first matmul: sq, lh16-k0,
            # rh16-k0, then the rest; rh8 last (consumed latest).
            sq_sb = wpool.tile([2, N], f16, tag="sq", name="sq")
            nc.sync.dma_start(out=sq_sb, in_=sq_d[:, :])
            lh16 = wpool.tile([128, KT, M], f16, tag="lh16", name="lh16")
            nc.sync.dma_start(out=lh16[:, 0:1, :], in_=lh16_d[:, 0:1, :])
            rh16 = wpool.tile([128, KT, N], f16, tag="rh16", name="rh16")
            nc.sync.dma_start(out=rh16[:, 0:1, :], in_=rh16_d[:, 0:1, :])
            nc.sync.dma_start(out=lh16[:, 1:, :], in_=lh16_d[:, 1:, :])
            for ki in range(1, KT):
                nc.sync.dma_start(out=rh16[:, ki:ki + 1, :],
                                  in_=rh16_d[:, ki:ki + 1, :])
            lh8 = wpool.tile([128, KT8, 2, 2, M], f8, tag="lh8", name="lh8")
            nc.sync.dma_start(out=lh8, in_=lh8_d[:, :, :, :, :])
            rh8 = wpool.tile([128, KT8, 2, 2, N], f8, tag="rh8", name="rh8")
            for ki in range(KT8):
                nc.sync.dma_start(out=rh8[:, ki], in_=rh8_d[:, ki])
            ones = wpool.tile([2, 128], f16, tag="ones", name="ones")
            nc.vector.memset(ones, 128.0)

            # one-time broadcast tile sqbc[p, j] = -sq[j]*2^11, built
            # during the DMA lead-in (PE idle anyway)
            sqbc = wpool.tile([128, N], f32, tag="sqbc", name="sqbc")
            ps = [psum.tile([128, 512], f32, tag="ps", name=f"psq{n}")
                  for n in range(NT)]
            for n in range(NT):
                nsl = slice(n * 512, (n + 1) * 512)
                nc.tensor.matmul(ps[n][:, :], ones, sq_sb[:, nsl],
                                 start=True, stop=True)
                nc.scalar.mul(sqbc[:, nsl], ps[n][:, :], 1.0)

            def hh(ps, rt, ki, n, start, stop=False):
                nc.tensor.matmul(
                    ps[n][:, :], lh16[:, ki, rt * 128:(rt + 1) * 128],
                    rh16[:, ki, n * 512:(n + 1) * 512],
                    start=start, stop=stop)

            def cross(ps, rt, ki, t, n, start=False, stop=False):
                # t=0: lo_i x hi_j ; t=1: hi_i x lo_j
                nc.tensor.matmul(
                    ps[n][:, :], lh8[:, ki, t, :, rt * 128:(rt + 1) * 128],
                    rh8[:, ki, 1 - t, :, n * 512:(n + 1) * 512],
                    start=start, stop=stop, perf_mode=DR)

            def drain(s_sb, ps, n):
                # s = psum + (-sq*2^11), fused on DVE
                nsl = slice(n * 512, (n + 1) * 512)
                nc.vector.scalar_tensor_tensor(
                    out=s_sb[:, nsl], in0=ps[n][:, :], scalar=1.0,
                    in1=sqbc[:, nsl], op0=mybir.AluOpType.mult,
                    op1=mybir.AluOpType.add)

            def topk(s_sb, m8, n):
                for h in range(2):
                    cs = slice(n * 512 + h * 256, n * 512 + (h + 1) * 256)
                    nc.vector.max(out=m8[:, n * 16 + h * 8:
                                         n * 16 + (h + 1) * 8],
                                  in_=s_sb[:, cs])

            def epilogue(s_sb, m8, rt):
                c8 = c8pool.tile([128, 8], f32, tag="c8", name="c8")
                m8x = m8pool.tile([128, 16 * 8], f32, tag="m8x", name="m8x")
                d8 = c8pool.tile([128, 8], f32, tag="d8", name="d8")
                nc.vector.max(out=c8, in_=m8)
                nc.vector.match_replace(out=m8x, in_to_replace=c8,
                                        in_values=m8, imm_value=NEG)
                nc.vector.max(out=d8, in_=m8x)
                sigma = d8[:, 7:8]

                H = N // 2
                mask = mpool.tile([128, H], bf16, tag="mask", name="mask")
                nc.vector.tensor_scalar(mask, s_sb[:, :H], sigma, None,
                                        op0=mybir.AluOpType.is_ge)
                nc.vector.scalar_tensor_tensor(
                    out=mask, in0=s_sb[:, H:], scalar=sigma, in1=mask,
                    op0=mybir.AluOpType.is_ge, op1=mybir.AluOpType.add)
                w = H // 2
                while w > S:
                    nc.vector.tensor_add(mask[:, :w], mask[:, :w],
                                         mask[:, w:2 * w])
                    w //= 2
                o = opool.tile([128, S], f32, tag="o", name="o")
                nc.vector.tensor_add(o, mask[:, :S], mask[:, S:2 * S])
                nc.sync.dma_start(
                    out=out_d[rt * 128:(rt + 1) * 128, :], in_=o)

            # ---- phase 1: rt0 hh-only paced by the rh16 stream; closed
            # early so rt1 can take the PSUM banks; cross backfills later.
            s0 = spool0.tile([128, N], f32, tag="s00", name="s00")
            ps = [psum.tile([128, 512], f32, tag="ps", name=f"ps{n}")
                  for n in range(NT)]
            for ki in range(KT):
                for n in range(NT):
                    hh(ps, 0, ki, n, start=(ki == 0), stop=(ki == KT - 1))
            for n in range(NT):
                drain(s0, ps, n)

            def full_rt(rt, s_sb, m8, n_outer=False):
                ps = [psum.tile([128, 512], f32, tag="ps", name=f"ps{n}")
                      for n in range(NT)]
                if n_outer:
                    for n in range(NT):
                        for ki in range(KT):
                            hh(ps, rt, ki, n, start=(ki == 0))
                        for ki in range(KT8):
                            for t in range(2):
                                cross(ps, rt, ki, t, n,
                                      stop=(ki == KT8 - 1 and t == 1))
                        drain(s_sb, ps, n)
                        topk(s_sb, m8, n)
                else:
                    for ki in range(KT):
                        for n in range(NT):
                            hh(ps, rt, ki, n, start=(ki == 0))
                    for ki in range(KT8):
                        for t in range(2):
                            for n in range(NT):
                                cross(ps, rt, ki, t, n,
                                      stop=(ki == KT8 - 1 and t == 1))
                    for n in range(NT):
                        drain(s_sb, ps, n)
                        topk(s_sb, m8, n)
                epilogue(s_sb, m8, rt)

            # ---- phase 2: rt1 fully fused (fp8 stream lands underneath)
            s1 = spool.tile([128, N], f32, tag="s", name="s_sb")
            m81 = m8pool.tile([128, 16 * 8], f32, tag="m8", name="m8")
            full_rt(1, s1, m81)

            # ---- phase 3: rt0 cross backfill; Act drains, GpSimd adds,
            # topk + epilogue chase under rt2's matmuls
            m80 = m8pool.tile([128, 16 * 8], f32, tag="m8", name="m8")
            ps = [psum.tile([128, 512], f32, tag="ps", name=f"ps{n}")
                  for n in range(NT)]
            for ki in range(KT8):
                for t in range(2):
                    for n in range(NT):
                        cross(ps, 0, ki, t, n, start=(ki == 0 and t == 0),
                              stop=(ki == KT8 - 1 and t == 1))
            for n in range(NT):
                nsl = slice(n * 512, (n + 1) * 512)
                tmp = tpool.tile([128, 512], f32, tag="tmp", name="tmp")
                nc.scalar.mul(tmp, ps[n][:, :], 1.0)
                nc.gpsimd.tensor_add(s0[:, nsl], s0[:, nsl], tmp)
                topk(s0, m80, n)
            epilogue(s0, m80, 0)

            # ---- phase 4/5: rt2 fused; rt3 fused n-outer (short tail)
            s2 = spool.tile([128, N], f32, tag="s", name="s_sb")
            m82 = m8pool.tile([128, 16 * 8], f32, tag="m8", name="m8")
            full_rt(2, s2, m82)
            s3 = spool.tile([128, N], f32, tag="s", name="s_sb")
            m83 = m8pool.tile([128, 16 * 8], f32, tag="m8", name="m8")
            full_rt(3, s3, m83, n_outer=True)

    nc.compile()
    return nc


def _prep_inputs(x):
    import ml_dtypes
    f8 = ml_dtypes.float8_e4m3

    flat = np.asarray(x, dtype=np.float32).reshape(N, D)
    sq = (flat.astype(np.float64) ** 2).sum(1)

    hi = flat.astype(np.float16)
    lo = (flat - hi.astype(np.float32)).astype(np.float16)

    rh16 = (hi.astype(np.float32) * 64.0).astype(np.float16)  # [N, D]
    hi8 = hi.astype(f8)
    lo8 = (lo.astype(np.float32) * 4096.0).astype(f8)

    # -16*sq rows (ones row is 2^7 -> product -sq*2^11)
    assert np.abs(sq).max() * 16.0 < 65000.0
    nsq_h = (-16.0 * sq).astype(np.float16)
    nsq_l = (-16.0 * sq - nsq_h.astype(np.float64)).astype(np.float16)
    sqrows = np.ascontiguousarray(np.stack([nsq_h, nsq_l]))  # [2, N]

    # [128, KT, N]: [p, ki, j] = hi*64 [j, ki*128+p]
    r16 = np.ascontiguousarray(rh16.T.reshape(KT, 128, N).transpose(1, 0, 2))
    # [128, KT8, 2, 2, N]: [p, ki, t, sub, j] = (lo8,hi8)[t][j, ki*256+sub*128+p]
    l8 = lo8.T.reshape(KT8, 2, 128, N)
    h8 = hi8.T.reshape(KT8, 2, 128, N)
    r8 = np.ascontiguousarray(
        np.stack([l8, h8], axis=1).transpose(3, 0, 1, 2, 4))

    in_maps = []
    for c in range(NCORES):
        rsl = slice(c * M, (c + 1) * M)
        in_maps.append({
            "rh16": r16, "rh8": r8, "sqrows": sqrows,
            "lh16": np.ascontiguousarray(r16[:, :, rsl]),
            "lh8": np.ascontiguousarray(r8[:, :, :, :, rsl]),
        })
    return in_maps


def kernel(x, k):
    assert int(k) == 16
    in_maps = _prep_inputs(x)

    if "nc" not in _cache:
        _cache["nc"] = _build()
    nc = _cache["nc"]

    from concourse.bass_utils import run_bass_kernel_spmd
    trace = bool(os.environ.get("KNN_TRACE"))
    if trace:
        try:
            from antenv.axon_hooks import get_axon_ntff_profile_hook
        except ImportError:
            trace = False
        else:
            trace = get_axon_ntff_profile_hook() is not None
    res = run_bass_kernel_spmd(nc, in_maps, core_ids=list(range(NCORES)),
                               trace=trace)
    _cache["res"] = res
    if trace and res.exec_time_ns is not None:
        print(f"HW exec time: {res.exec_time_ns} ns")
        _cache["exec_time_ns"] = res.exec_time_ns

    out = np.concatenate([r["out"] for r in res.results], axis=0)
    return out.reshape(B, S, S)


# revision 9
# speedup vs baseline: 1.1269x; 1.0864x over previous
"""kNN hypergraph kernel for Trainium2 (8 NeuronCores, Bass/Tile).

Problem: x [16, 256, 768] f32, k=16.
  flat = x.reshape(4096, 768)
  d2[i,j] = |flat_i - flat_j|^2 ; idx = 16 nearest (incl self)
  hypergraph[i, idx[i,:]] = 1 ; out[b,s,t] = sum_b2 hg[b*256+s, b2*256+t]
Output: [16, 256, 256] f32 (per-row histogram of neighbor_index % 256).

Strategy (row-sharded across 8 cores, 512 rows each):
  - Rank rows by s[i,j] = <x_i,x_j> - |x_j|^2/2 (uniform halving of
    2<x_i,x_j> - |x_j|^2; the per-row constant sq_i does not change the
    ranking). The 16 NN are the 16 LARGEST s per row.
  - Exact-match numerics via a hi/lo fp16+fp8 split at a 2^12 PSUM scale:
      hh:    (2^6 hi)^T (2^6 hi)          fp16, 6 K-tiles of 128
      cross: (2^12 lo8)^T hi8 + hi8^T (2^12 lo8)
                                          fp8 e4m3 DoubleRow, 2x3 K-tiles
    12 matmul-tiles per (row-tile, col-block); every matmul costs the
    same 216ns (512 output cols at 1 col/cycle), so slots are the PE
    currency. The -sq*2^11 term is added during the PSUM->SBUF drain
    (DVE scalar_tensor_tensor against a broadcast tile built once at
    startup by eight K=2 ones x [-16sq_h; -16sq_l] matmuls).
  - DMA layout: streamed tensors are packed partition-major with >=8KB
    contiguous per partition so the DMA queue runs at line rate
    (~420 GB/s measured) instead of descriptor-bound (~112 GB/s at
    2.7KB packets); per-K-tile chunks keep the first matmul start early.
  - Top-16 per row: per 256-column chunk one DVE max8 -> chunk top-8;
    combine -> sigma = 16th largest; mask (s >= sigma) fused with the
    first fold, then binary-tree adds fold the 16 blocks of 256.
  - Phasing: rt0 hh paced by the rh16 stream (closed early, partial
    sums drained); rt1 runs fully fused while the fp8 stream lands;
    rt0's cross then backfills into fresh PSUM groups (Act drain +
    GpSimd add); rt2/rt3 run fully fused, rt3 n-outer so its drain/topk
    chase per column block, shrinking the tail.
"""

import os

import numpy as np

B, S, D = 16, 256, 768
N = B * S            # 4096 points
NCORES = 8
M = N // NCORES      # 512 rows per core
KT = 6               # fp16 K tiles of 128 (768 features)
KT8 = 3              # fp8 DoubleRow K tiles of 256
NT = N // 512        # 8 moving tiles of 512 columns
RT = M // 128        # 4 row-tiles of 128 per core
NEG = -3.0e38        # sentinel: far below any real s value

_cache = {}


def _build():
    import concourse.mybir as mybir
    import concourse.tile as tile
    from concourse import bacc

    f32 = mybir.dt.float32
    f16 = mybir.dt.float16
    bf16 = mybir.dt.bfloat16
    f8 = mybir.dt.float8e4
    DR = mybir.MatmulPerfMode.DoubleRow

    nc = bacc.Bacc("TRN2", target_bir_lowering=False, debug=False,
                   num_devices=NCORES)

    rh16_d = nc.dram_tensor("rh16", [128, KT, N], f16, kind="ExternalInput")
    rh8_d = nc.dram_tensor("rh8", [128, KT8, 2, 2, N], f8,
                           kind="ExternalInput")
    lh16_d = nc.dram_tensor("lh16", [128, KT, M], f16, kind="ExternalInput")
    lh8_d = nc.dram_tensor("lh8", [128, KT8, 2, 2, M], f8,
                           kind="ExternalInput")
    sq_d = nc.dram_tensor("sqrows", [2, N], f16, kind="ExternalInput")
    out_d = nc.dram_tensor("out", [M, S], f32, kind="ExternalOutput")

    with tile.TileContext(nc) as tc:
        with (
            tc.tile_pool(name="weights", bufs=1) as wpool,
            tc.tile_pool(name="s", bufs=2) as spool,
            tc.tile_pool(name="s01", bufs=1) as spool0,
            tc.tile_pool(name="tmp", bufs=2) as tpool,
            tc.tile_pool(name="mask", bufs=2) as mpool,
            tc.tile_pool(name="m8", bufs=2) as m8pool,
            tc.tile_pool(name="c8", bufs=4) as c8pool,
            tc.tile_pool(name="outp", bufs=4) as opool,
            tc.tile_pool(name="psum", bufs=8, space="PSUM") as psum,
        ):
            # DMA order tuned for earliest first matmul: sq, lh16-k0,
            # rh16-k0, then the rest; rh8 last (consumed latest).
            sq_sb = wpool.tile([2, N], f16, tag="sq", name="sq")
            nc.sync.dma_start(out=sq_sb, in_=sq_d[:, :])
            lh16 = wpool.tile([128, KT, M], f16, tag="lh16", name="lh16")
            nc.sync.dma_start(out=lh16[:, 0:1, :], in_=lh16_d[:, 0:1, :])
            rh16 = wpool.tile([128, KT, N], f16, tag="rh16", name="rh16")
            nc.sync.dma_start(out=rh16[:, 0:1, :], in_=rh16_d[:, 0:1, :])
            nc.sync.dma_start(out=lh16[:, 1:, :], in_=lh16_d[:, 1:, :])
            for ki in range(1, KT):
                nc.sync.dma_start(out=rh16[:, ki:ki + 1, :],
                                  in_=rh16_d[:, ki:ki + 1, :])
            lh8 = wpool.tile([128, KT8, 2, 2, M], f8, tag="lh8", name="lh8")
            nc.sync.dma_start(out=lh8, in_=lh8_d[:, :, :, :, :])
            rh8 = wpool.tile([128, KT8, 2, 2, N], f8, tag="rh8", name="rh8")
            for ki in range(KT8):
                nc.sync.dma_start(out=rh8[:, ki], in_=rh8_d[:, ki])
            ones = wpool.tile([2, 128], f16, tag="ones", name="ones")
            nc.vector.memset(ones, 128.0)

            # one-time broadcast tile sqbc[p, j] = -sq[j]*2^11, built
            # during the DMA lead-in (PE idle anyway)
            sqbc = wpool.tile([128, N], f32, tag="sqbc", name="sqbc")
            ps = [psum.tile([128, 512], f32, tag="ps", name=f"psq{n}")
                  for n in range(NT)]
            for n in range(NT):
                nsl = slice(n * 512, (n + 1) * 512)
                nc.tensor.matmul(ps[n][:, :], ones, sq_sb[:, nsl],
                                 start=True, stop=True)
                nc.scalar.mul(sqbc[:, nsl], ps[n][:, :], 1.0)

            def hh(ps, rt, ki, n, start, stop=False):
                nc.tensor.matmul(
                    ps[n][:, :], lh16[:, ki, rt * 128:(rt + 1) * 128],
                    rh16[:, ki, n * 512:(n + 1) * 512],
                    start=start, stop=stop)

            def cross(ps, rt, ki, t, n, start=False, stop=False):
                # t=0: lo_i x hi_j ; t=1: hi_i x lo_j
                nc.tensor.matmul(
                    ps[n][:, :], lh8[:, ki, t, :, rt * 128:(rt + 1) * 128],
                    rh8[:, ki, 1 - t, :, n * 512:(n + 1) * 512],
                    start=start, stop=stop, perf_mode=DR)

            def drain(s_sb, ps, n):
                # s = psum + (-sq*2^11), fused on DVE
                nsl = slice(n * 512, (n + 1) * 512)
                nc.vector.scalar_tensor_tensor(
                    out=s_sb[:, nsl], in0=ps[n][:, :], scalar=1.0,
                    in1=sqbc[:, nsl], op0=mybir.AluOpType.mult,
                    op1=mybir.AluOpType.add)

            def topk(s_sb, m8, n):
                for h in range(2):
                    cs = slice(n * 512 + h * 256, n * 512 + (h + 1) * 256)
                    nc.vector.max(out=m8[:, n * 16 + h * 8:
                                         n * 16 + (h + 1) * 8],
                                  in_=s_sb[:, cs])

            def epilogue(s_sb, m8, rt):
                c8 = c8pool.tile([128, 8], f32, tag="c8", name="c8")
                m8x = m8pool.tile([128, 16 * 8], f32, tag="m8x", name="m8x")
                d8 = c8pool.tile([128, 8], f32, tag="d8", name="d8")
                nc.vector.max(out=c8, in_=m8)
                nc.vector.match_replace(out=m8x, in_to_replace=c8,
                                        in_values=m8, imm_value=NEG)
                nc.vector.max(out=d8, in_=m8x)
                sigma = d8[:, 7:8]

                H = N // 2
                mask = mpool.tile([128, H], bf16, tag="mask", name="mask")
                nc.vector.tensor_scalar(mask, s_sb[:, :H], sigma, None,
                                        op0=mybir.AluOpType.is_ge)
                nc.vector.scalar_tensor_tensor(
                    out=mask, in0=s_sb[:, H:], scalar=sigma, in1=mask,
                    op0=mybir.AluOpType.is_ge, op1=mybir.AluOpType.add)
                w = H // 2
                while w > S:
                    nc.vector.tensor_add(mask[:, :w], mask[:, :w],
                                         mask[:, w:2 * w])
                    w //= 2
                o = opool.tile([128, S], f32, tag="o", name="o")
                nc.vector.tensor_add(o, mask[:, :S], mask[:, S:2 * S])
                nc.sync.dma_start(
                    out=out_d[rt * 128:(rt + 1) * 128, :], in_=o)

            # ---- phase 1: rt0 hh-only paced by the rh16 stream; closed
            # early so rt1 can take the PSUM banks; cross backfills later.
            s0 = spool0.tile([128, N], f32, tag="s00", name="s00")
            ps = [psum.tile([128, 512], f32, tag="ps", name=f"ps{n}")
                  for n in range(NT)]
            for ki in range(KT):
                for n in range(NT):
                    hh(ps, 0, ki, n, start=(ki == 0), stop=(ki == KT - 1))
            for n in range(NT):
                drain(s0, ps, n)

            def sq_close(ps, n, stop=True):
                nsl = slice(n * 512, (n + 1) * 512)
                nc.tensor.matmul(ps[n][:, :], ones, sq_sb[:, nsl],
                                 start=False, stop=stop)

            def full_rt(rt, s_sb, m8, n_outer=False, act_drain=False):
                # act_drain: sq via a 13th matmul + Act drain, keeping DVE
                # free in this row-tile's chase era (used for the final
                # tiles where DVE backlog would otherwise become the tail)
                ps = [psum.tile([128, 512], f32, tag="ps", name=f"ps{n}")
                      for n in range(NT)]
                if n_outer:
                    for n in range(NT):
                        for ki in range(KT):
                            hh(ps, rt, ki, n, start=(ki == 0))
                        for ki in range(KT8):
                            for t in range(2):
                                cross(ps, rt, ki, t, n,
                                      stop=(not act_drain and
                                            ki == KT8 - 1 and t == 1))
                        if act_drain:
                            sq_close(ps, n)
                            nc.scalar.mul(s_sb[:, n * 512:(n + 1) * 512],
                                          ps[n][:, :], 1.0)
                        else:
                            drain(s_sb, ps, n)
                        topk(s_sb, m8, n)
                else:
                    for ki in range(KT):
                        for n in range(NT):
                            hh(ps, rt, ki, n, start=(ki == 0))
                    for ki in range(KT8):
                        for t in range(2):
                            for n in range(NT):
                                cross(ps, rt, ki, t, n,
                                      stop=(not act_drain and
                                            ki == KT8 - 1 and t == 1))
                    for n in range(NT):
                        if act_drain:
                            sq_close(ps, n)
                            nc.scalar.mul(s_sb[:, n * 512:(n + 1) * 512],
                                          ps[n][:, :], 1.0)
                        else:
                            drain(s_sb, ps, n)
                        topk(s_sb, m8, n)
                epilogue(s_sb, m8, rt)

            # ---- phase 2: rt1 fully fused (fp8 stream lands underneath)
            s1 = spool.tile([128, N], f32, tag="s", name="s_sb")
            m81 = m8pool.tile([128, 16 * 8], f32, tag="m8", name="m8")
            full_rt(1, s1, m81)

            # ---- phase 3: rt0 cross backfill; Act drains, GpSimd adds,
            # topk + epilogue chase under rt2's matmuls
            m80 = m8pool.tile([128, 16 * 8], f32, tag="m8", name="m8")
            ps = [psum.tile([128, 512], f32, tag="ps", name=f"ps{n}")
                  for n in range(NT)]
            # n-outer so each bank's drain chases right behind its 6
            # matmuls (K-outer here stalls the PE on bank frees)
            for n in range(NT):
                for ki in range(KT8):
                    for t in range(2):
                        cross(ps, 0, ki, t, n, start=(ki == 0 and t == 0),
                              stop=(ki == KT8 - 1 and t == 1))
                nsl = slice(n * 512, (n + 1) * 512)
                tmp = tpool.tile([128, 512], f32, tag="tmp", name="tmp")
                nc.scalar.mul(tmp, ps[n][:, :], 1.0)
                nc.gpsimd.tensor_add(s0[:, nsl], s0[:, nsl], tmp)
                topk(s0, m80, n)
            epilogue(s0, m80, 0)

            # ---- phase 4/5: rt2 fused; rt3 fused n-outer (short tail)
            s2 = spool.tile([128, N], f32, tag="s", name="s_sb")
            m82 = m8pool.tile([128, 16 * 8], f32, tag="m8", name="m8")
            full_rt(2, s2, m82, act_drain=True)
            s3 = spool.tile([128, N], f32, tag="s", name="s_sb")
            m83 = m8pool.tile([128, 16 * 8], f32, tag="m8", name="m8")
            full_rt(3, s3, m83, n_outer=True, act_drain=True)

    nc.compile()
    return nc


def _prep_inputs(x):
    import ml_dtypes
    f8 = ml_dtypes.float8_e4m3

    flat = np.asarray(x, dtype=np.float32).reshape(N, D)
    sq = (flat.astype(np.float64) ** 2).sum(1)

    hi = flat.astype(np.float16)
    lo = (flat - hi.astype(np.float32)).astype(np.float16)

    rh16 = (hi.astype(np.float32) * 64.0).astype(np.float16)  # [N, D]
    hi8 = hi.astype(f8)
    lo8 = (lo.astype(np.float32) * 4096.0).astype(f8)

    # -16*sq rows (ones row is 2^7 -> product -sq*2^11)
    assert np.abs(sq).max() * 16.0 < 65000.0
    nsq_h = (-16.0 * sq).astype(np.float16)
    nsq_l = (-16.0 * sq - nsq_h.astype(np.float64)).astype(np.float16)
    sqrows = np.ascontiguousarray(np.stack([nsq_h, nsq_l]))  # [2, N]

    # [128, KT, N]: [p, ki, j] = hi*64 [j, ki*128+p]
    r16 = np.ascontiguousarray(rh16.T.reshape(KT, 128, N).transpose(1, 0, 2))
    # [128, KT8, 2, 2, N]: [p, ki, t, sub, j] = (lo8,hi8)[t][j, ki*256+sub*128+p]
    l8 = lo8.T.reshape(KT8, 2, 128, N)
    h8 = hi8.T.reshape(KT8, 2, 128, N)
    r8 = np.ascontiguousarray(
        np.stack([l8, h8], axis=1).transpose(3, 0, 1, 2, 4))

    in_maps = []
    for c in range(NCORES):
        rsl = slice(c * M, (c + 1) * M)
        in_maps.append({
            "rh16": r16, "rh8": r8, "sqrows": sqrows,
            "lh16": np.ascontiguousarray(r16[:, :, rsl]),
            "lh8": np.ascontiguousarray(r8[:, :, :, :, rsl]),
        })
    return in_maps


def kernel(x, k):
    assert int(k) == 16
    in_maps = _prep_inputs(x)

    if "nc" not in _cache:
        _cache["nc"] = _build()
    nc = _cache["nc"]

    from concourse.bass_utils import run_bass_kernel_spmd
    trace = bool(os.environ.get("KNN_TRACE"))
    if trace:
        try:
            from antenv.axon_hooks import get_axon_ntff_profile_hook
        except ImportError:
            trace = False
        else:
            trace = get_axon_ntff_profile_hook() is not None
    res = run_bass_kernel_spmd(nc, in_maps, core_ids=list(range(NCORES)),
                               trace=trace)
    _cache["res"] = res
    if trace and res.exec_time_ns is not None:
        print(f"HW exec time: {res.exec_time_ns} ns")
        _cache["exec_time_ns"] = res.exec_time_ns

    out = np.concatenate([r["out"] for r in res.results], axis=0)
    return out.reshape(B, S, S)
